# revision 4
# baseline (speedup 1.0000x reference)
"""CrossModalAttention on 8 Trainium2 NeuronCores (Bass/Tile, SPMD).

Sharding: data-parallel over batch B=8 (one batch element per core),
weights replicated. Each core computes, for its batch element:

  fp_i = relu(f_i @ Wp_i^T + bp_i)            i in {1,2,3}
  q_i, k_i = fp_i @ Wq_i^T, fp_i @ Wk_i^T ;  v_i = fp_i @ Wv_i^T
  s1 = (q2+q3) k1^T ; c13 = q1 k3^T ; s2 = c13 + q3 k2^T ; s3 = c13 + q2 k3^T
  u_i = softmax(s_i) v_i
  out = concat([u1, u2, u3, fp1, fp2, fp3], -1)

Device layout: activations live transposed ([D, T], contraction dim on
SBUF partitions); scores come out natural [q, k] for free-axis softmax;
p is PE-transposed for the AV matmul. Matmuls run as float32r (fp32 data,
single-pass PE mode); softmax & accumulation in fp32.

Host pre-layout: f_i^T, and the q/k/p weights pre-swizzled so each
stationary [128,128] lhsT chunk is a contiguous DRAM row-block.
"""

import math
from contextlib import ExitStack

import numpy as np

FP = None  # set in _lazy_imports
_STATE = {}

B, T, D = 8, 1024, 1024
P = 128
MM_DTYPE_NAME = "float32r"


def _lazy_imports():
    global tile, bacc, mybir, bass, make_identity, FP, AF
    import concourse.bass as bass
    import concourse.tile as tile
    from concourse import bacc, mybir
    from concourse.masks import make_identity
    FP = mybir.dt.float32
    AF = mybir.ActivationFunctionType


def build_nc(T=T, D=D, reps=1, mm_dtype_name=MM_DTYPE_NAME):
    _lazy_imports()
    mm_dtype = getattr(mybir.dt, mm_dtype_name)
    MDT = mm_dtype
    tr_dtype = mm_dtype
    Cd = D // P
    Ct = T // P
    S = min(512, T)
    NS = T // S
    Sv = min(512, D)
    NSv = D // Sv

    nc = bacc.Bacc("TRN2", target_bir_lowering=False, debug=False,
                   enable_asserts=False, num_devices=8)

    fT, Wp_s, Wq_s, Wk_s, WvT, bp = [], [], [], [], [], []
    qTd, vd, pTd = [], [], []
    for i in range(3):
        fT.append(nc.dram_tensor(f"fT{i}", [D, T], MDT, kind="ExternalInput").ap())
        Wp_s.append(nc.dram_tensor(f"Wp_s{i}", [D, D], MDT, kind="ExternalInput").ap())
        Wq_s.append(nc.dram_tensor(f"Wq_s{i}", [D, D], MDT, kind="ExternalInput").ap())
        Wk_s.append(nc.dram_tensor(f"Wk_s{i}", [D, D], MDT, kind="ExternalInput").ap())
        WvT.append(nc.dram_tensor(f"WvT{i}", [D, D], MDT, kind="ExternalInput").ap())
        bp.append(nc.dram_tensor(f"bp{i}", [D], FP, kind="ExternalInput").ap())
        qTd.append(nc.dram_tensor(f"qTd{i}", [D, T], MDT, kind="Internal").ap())
        vd.append(nc.dram_tensor(f"vd{i}", [T, D], MDT, kind="Internal").ap())
        pTd.append(nc.dram_tensor(f"pTd{i}", [Ct, T, P], MDT, kind="Internal").ap())
    out = nc.dram_tensor("out", [T, 6 * D], FP, kind="ExternalOutput").ap()

    def mm(ps, lhsT, rhs, start, stop):
        nc.tensor.matmul(ps, lhsT, rhs, start=start, stop=stop)

    with tile.TileContext(nc) as tc:
        with ExitStack() as top:
            const = top.enter_context(tc.tile_pool(name="const", bufs=1))
            ident_f = const.tile([P, P], FP)
            make_identity(nc, ident_f[:])
            ident = const.tile([P, P], tr_dtype)
            nc.vector.tensor_copy(ident[:], ident_f[:])
            bias_t = const.tile([P, 3 * Cd], FP)
            for i in range(3):
                nc.sync.dma_start(bias_t[:, i * Cd:(i + 1) * Cd],
                                  bp[i].rearrange("(c p) -> p c", p=P))
            for _rep in range(reps):
                _emit_body(nc, tc, mm, ident, bias_t,
                           fT, Wp_s, Wq_s, Wk_s, WvT, qTd, vd, pTd, out,
                           Cd, Ct, S, NS, Sv, NSv, tr_dtype, MDT, D, T)
    nc.compile()
    return nc


def _emit_body(nc, tc, mm, ident, bias_t, fT, Wp_s, Wq_s, Wk_s, WvT,
               qTd, vd, pTd, out, Cd, Ct, S, NS, Sv, NSv, tr_dtype, MDT, D, T):
    kt_tiles = [[None] * Cd for _ in range(3)]
    with ExitStack() as sAB1:
        ktp = sAB1.enter_context(tc.tile_pool(name="kt", bufs=3 * Cd))

        # ---------------- Stage A: projections, q/v spill, kT resident ----
        with ExitStack() as sA:
            ftp = sA.enter_context(tc.tile_pool(name="ft", bufs=Cd))
            fptp = sA.enter_context(tc.tile_pool(name="fpt", bufs=Cd))
            wsp = sA.enter_context(tc.tile_pool(name="wstream", bufs=2))
            wvp = sA.enter_context(tc.tile_pool(name="wv", bufs=Cd))
            qst = sA.enter_context(tc.tile_pool(name="qstage", bufs=2))
            vst = sA.enter_context(tc.tile_pool(name="vstage", bufs=2))
            fst = sA.enter_context(tc.tile_pool(name="fpstage", bufs=2))
            psA = sA.enter_context(tc.tile_pool(name="psA", bufs=4, space="PSUM"))
            psT = sA.enter_context(tc.tile_pool(name="psT", bufs=2, space="PSUM"))

            for i in range(3):
                ft_t = []
                for dc in range(Cd):
                    t = ftp.tile([P, T], MDT, tag="ft", name="ft")
                    nc.sync.dma_start(t[:], fT[i][dc * P:(dc + 1) * P, :])
                    ft_t.append(t)
                # fpT_i = relu(Wp f^T + b)
                fpt_t = [fptp.tile([P, T], MDT, tag="fpt", name="fpt")
                         for _ in range(Cd)]
                for ec in range(Cd):
                    w = wsp.tile([P, Cd * P], MDT, tag="w")
                    nc.sync.dma_start(w[:], Wp_s[i][ec * P:(ec + 1) * P, :])
                    for tn in range(NS):
                        ps = psA.tile([P, S], FP, tag="psA")
                        for dc in range(Cd):
                            mm(ps[:], w[:, dc * P:(dc + 1) * P],
                               ft_t[dc][:, tn * S:(tn + 1) * S],
                               dc == 0, dc == Cd - 1)
                        nc.scalar.activation(
                            fpt_t[ec][:, tn * S:(tn + 1) * S], ps[:], AF.Relu,
                            bias=bias_t[:, i * Cd + ec:i * Cd + ec + 1])
                # fp_i natural -> out[:, 3D + i*D ...]
                for tb in range(Ct):
                    fs = fst.tile([P, D], FP, tag="fps")
                    for ec in range(Cd):
                        pt = psT.tile([P, P], tr_dtype, tag="psT")
                        nc.tensor.transpose(
                            pt[:],
                            fpt_t[ec][:, tb * P:(tb + 1) * P],
                            ident[:])
                        nc.vector.tensor_copy(
                            fs[:, ec * P:(ec + 1) * P], pt[:].bitcast(FP))
                    nc.sync.dma_start(
                        out[tb * P:(tb + 1) * P, 3 * D + i * D:3 * D + (i + 1) * D],
                        fs[:])
                # qT_i -> DRAM
                for oc in range(Cd):
                    w = wsp.tile([P, Cd * P], MDT, tag="w")
                    nc.sync.dma_start(w[:], Wq_s[i][oc * P:(oc + 1) * P, :])
                    qs = qst.tile([P, T], MDT, tag="qs")
                    for tn in range(NS):
                        ps = psA.tile([P, S], FP, tag="psA")
                        for ec in range(Cd):
                            mm(ps[:], w[:, ec * P:(ec + 1) * P],
                               fpt_t[ec][:, tn * S:(tn + 1) * S],
                               ec == 0, ec == Cd - 1)
                        nc.vector.tensor_copy(qs[:, tn * S:(tn + 1) * S], ps[:])
                    nc.sync.dma_start(qTd[i][oc * P:(oc + 1) * P, :], qs[:])
                # v_i -> DRAM (Wv^T streamed by output-column block)
                for en in range(NSv):
                    wv_t = []
                    for ec in range(Cd):
                        t = wvp.tile([P, Sv], MDT, tag="wv", name="wv")
                        nc.sync.dma_start(
                            t[:], WvT[i][ec * P:(ec + 1) * P, en * Sv:(en + 1) * Sv])
                        wv_t.append(t)
                    for tb in range(Ct):
                        ps = psA.tile([P, Sv], FP, tag="psA")
                        for ec in range(Cd):
                            mm(ps[:], fpt_t[ec][:, tb * P:(tb + 1) * P],
                               wv_t[ec][:], ec == 0, ec == Cd - 1)
                        vs = vst.tile([P, Sv], MDT, tag="vs")
                        nc.vector.tensor_copy(vs[:], ps[:])
                        nc.sync.dma_start(
                            vd[i][tb * P:(tb + 1) * P, en * Sv:(en + 1) * Sv], vs[:])
                # kT_i -> SBUF resident
                for oc in range(Cd):
                    w = wsp.tile([P, Cd * P], MDT, tag="w")
                    nc.sync.dma_start(w[:], Wk_s[i][oc * P:(oc + 1) * P, :])
                    kt = ktp.tile([P, T], MDT, tag="kt")
                    for tn in range(NS):
                        ps = psA.tile([P, S], FP, tag="psA")
                        for ec in range(Cd):
                            mm(ps[:], w[:, ec * P:(ec + 1) * P],
                               fpt_t[ec][:, tn * S:(tn + 1) * S],
                               ec == 0, ec == Cd - 1)
                        nc.scalar.copy(kt[:, tn * S:(tn + 1) * S], ps[:])
                    kt_tiles[i][oc] = kt

        # ---------------- Stage B1: scores + softmax + p^T spill ----------
        with ExitStack() as sB1:
            qlp = sB1.enter_context(tc.tile_pool(name="qload", bufs=6))
            ppp = sB1.enter_context(tc.tile_pool(name="pp", bufs=3))
            ptp = sB1.enter_context(tc.tile_pool(name="ptstage", bufs=3))
            c13p = sB1.enter_context(tc.tile_pool(name="c13", bufs=2))
            stp = sB1.enter_context(tc.tile_pool(name="stats", bufs=2))
            psS = sB1.enter_context(tc.tile_pool(name="psS", bufs=3, space="PSUM"))
            psT2 = sB1.enter_context(tc.tile_pool(name="psT2", bufs=2, space="PSUM"))

            for qb in range(Ct):
                qt = []
                for i in range(3):
                    q = qlp.tile([P, Cd * P], MDT, tag="q", name="q")
                    nc.sync.dma_start(
                        q[:].rearrange("p (c t) -> p c t", c=Cd),
                        qTd[i].rearrange("(c p) t -> p c t", p=P)
                           [:, :, qb * P:(qb + 1) * P])
                    qt.append(q)
                stats = stp.tile([P, 16], FP, tag="stats")

                def softmax(s_ps, i_mod, col, qb=qb, stats=stats):
                    negmax = stats[:, col:col + 1]
                    denom = stats[:, col + 4:col + 5]
                    recip = stats[:, col + 8:col + 9]
                    nc.vector.tensor_reduce(negmax, s_ps[:], mybir.AxisListType.X,
                                            mybir.AluOpType.max, negate=True)
                    p = ppp.tile([P, T], MDT, tag="p", name="p")
                    nc.scalar.activation(p[:], s_ps[:], AF.Exp,
                                         bias=negmax, accum_out=denom)
                    nc.vector.reciprocal(recip, denom)
                    nc.vector.tensor_scalar_mul(p[:], p[:], recip)
                    pts = ptp.tile([P, T], MDT, tag="pts", name="pts")
                    for kc in range(Ct):
                        pt = psT2.tile([P, P], tr_dtype, tag="psT2", name="pt")
                        nc.tensor.transpose(
                            pt[:], p[:, kc * P:(kc + 1) * P],
                            ident[:])
                        nc.vector.tensor_copy(
                            pts[:, kc * P:(kc + 1) * P], pt[:])
                    nc.sync.dma_start(
                        pTd[i_mod][qb].rearrange("(kc p) q -> p kc q", p=P),
                        pts[:].rearrange("p (kc q) -> p kc q", q=P))

                # s1 = (q2+q3) k1^T
                s1 = psS.tile([P, T], FP, tag="s", name="s1")
                for nh in range(NS):
                    for ec in range(Cd):
                        mm(s1[:, nh * S:(nh + 1) * S], qt[1][:, ec * P:(ec + 1) * P],
                           kt_tiles[0][ec][:, nh * S:(nh + 1) * S], ec == 0, False)
                    for ec in range(Cd):
                        mm(s1[:, nh * S:(nh + 1) * S], qt[2][:, ec * P:(ec + 1) * P],
                           kt_tiles[0][ec][:, nh * S:(nh + 1) * S], False,
                           ec == Cd - 1)
                softmax(s1, 0, 0)
                # c13 = q1 k3^T
                c13ps = psS.tile([P, T], FP, tag="s", name="c13ps")
                for nh in range(NS):
                    for ec in range(Cd):
                        mm(c13ps[:, nh * S:(nh + 1) * S],
                           qt[0][:, ec * P:(ec + 1) * P],
                           kt_tiles[2][ec][:, nh * S:(nh + 1) * S], ec == 0,
                           ec == Cd - 1)
                c13 = c13p.tile([P, T], FP, tag="c13", name="c13")
                nc.vector.tensor_copy(c13[:], c13ps[:])
                # s2 = c13 + q3 k2^T
                s2 = psS.tile([P, T], FP, tag="s", name="s2")
                for nh in range(NS):
                    for ec in range(Cd):
                        mm(s2[:, nh * S:(nh + 1) * S], qt[2][:, ec * P:(ec + 1) * P],
                           kt_tiles[1][ec][:, nh * S:(nh + 1) * S], ec == 0,
                           ec == Cd - 1)
                nc.vector.tensor_tensor(s2[:], s2[:], c13[:], mybir.AluOpType.add)
                softmax(s2, 1, 1)
                # s3 = c13 + q2 k3^T
                s3 = psS.tile([P, T], FP, tag="s", name="s3")
                for nh in range(NS):
                    for ec in range(Cd):
                        mm(s3[:, nh * S:(nh + 1) * S], qt[1][:, ec * P:(ec + 1) * P],
                           kt_tiles[2][ec][:, nh * S:(nh + 1) * S], ec == 0,
                           ec == Cd - 1)
                nc.vector.tensor_tensor(s3[:], s3[:], c13[:], mybir.AluOpType.add)
                softmax(s3, 2, 2)

    # ---------------- Stage B2: u = p v -------------------------------
    with ExitStack() as sB2:
        vlp = sB2.enter_context(tc.tile_pool(name="vload", bufs=3 * Ct))
        ptl = sB2.enter_context(tc.tile_pool(name="ptload", bufs=6))
        ust = sB2.enter_context(tc.tile_pool(name="ustage", bufs=3))
        psU = sB2.enter_context(tc.tile_pool(name="psU", bufs=3, space="PSUM"))

        v_tiles = [[None] * Ct for _ in range(3)]
        for i in range(3):
            for kc in range(Ct):
                t = vlp.tile([P, D], MDT, tag="v", name="v")
                nc.sync.dma_start(t[:], vd[i][kc * P:(kc + 1) * P, :])
                v_tiles[i][kc] = t
        for qb in range(Ct):
            for i in range(3):
                ptile = ptl.tile([P, T], MDT, tag="pt", name="ptile")
                nc.sync.dma_start(
                    ptile[:].rearrange("p (kc q) -> p kc q", q=P),
                    pTd[i][qb].rearrange("(kc p) q -> p kc q", p=P))
                psu = psU.tile([P, D], FP, tag="psu")
                for en in range(NSv):
                    for kc in range(Ct):
                        mm(psu[:, en * Sv:(en + 1) * Sv],
                           ptile[:, kc * P:(kc + 1) * P],
                           v_tiles[i][kc][:, en * Sv:(en + 1) * Sv],
                           kc == 0, kc == Ct - 1)
                us = ust.tile([P, D], FP, tag="us")
                nc.scalar.copy(us[:], psu[:])
                nc.sync.dma_start(out[qb * P:(qb + 1) * P, i * D:(i + 1) * D], us[:])


# ---------------------------------------------------------------------------
# Host side: runner + kernel()
# ---------------------------------------------------------------------------

def _make_runner(nc, n_cores=8):
    import jax
    from jax.sharding import Mesh, PartitionSpec
    from jax.experimental.shard_map import shard_map
    from concourse import mybir
    from concourse.bass2jax import (_bass_exec_p, install_neuronx_cc_hook,
                                    partition_id_tensor)

    install_neuronx_cc_hook()
    partition_name = (nc.partition_id_tensor.name
                      if nc.partition_id_tensor else None)
    in_names, out_names, out_avals, zero_outs = [], [], [], []
    for alloc in nc.m.functions[0].allocations:
        if not isinstance(alloc, mybir.MemoryLocationSet):
            continue
        name = alloc.memorylocations[0].name
        if alloc.kind == "ExternalInput":
            if name != partition_name:
                in_names.append(name)
        elif alloc.kind == "ExternalOutput":
            out_names.append(name)
            shape = tuple(alloc.tensor_shape)
            dtype = mybir.dt.np(alloc.dtype)
            out_avals.append(jax.core.ShapedArray(shape, dtype))
            zero_outs.append(np.zeros(shape, dtype))
    n_params = len(in_names)
    all_names = in_names + out_names
    if partition_name is not None:
        all_names.append(partition_name)

    def _body(*args):
        operands = list(args)
        if partition_name is not None:
            operands.append(partition_id_tensor())
        outs = _bass_exec_p.bind(
            *operands,
            out_avals=tuple(out_avals),
            in_names=tuple(all_names),
            out_names=tuple(out_names),
            lowering_input_output_aliases=(),
            sim_require_finite=True,
            sim_require_nnan=True,
            nc=nc,
        )
        return tuple(outs)

    devices = jax.devices()[:n_cores]
    mesh = Mesh(np.asarray(devices), ("core",))
    specs = (PartitionSpec("core"),)
    sharded = jax.jit(
        shard_map(_body, mesh=mesh,
                  in_specs=specs * (n_params + len(out_names)),
                  out_specs=specs * len(out_names), check_rep=False),
        keep_unused=True,
    )
    sharding = jax.sharding.NamedSharding(mesh, PartitionSpec("core"))

    def prepare(in_maps):
        per_core = [[np.asarray(m[name]) for name in in_names] for m in in_maps]
        concat_in = [np.concatenate([per_core[c][i] for c in range(n_cores)],
                                    axis=0) for i in range(n_params)]
        concat_zeros = [np.zeros((n_cores * z.shape[0], *z.shape[1:]), z.dtype)
                        for z in zero_outs]
        dev_in = [jax.device_put(a, sharding) for a in concat_in]
        dev_zero = [jax.device_put(a, sharding) for a in concat_zeros]
        jax.block_until_ready(dev_in)
        jax.block_until_ready(dev_zero)

        def execute():
            out = sharded(*dev_in, *dev_zero)
            jax.block_until_ready(out)
            return out

        def fetch(out):
            return [
                {name: np.asarray(out[i]).reshape(n_cores, *out_avals[i].shape)[c]
                 for i, name in enumerate(out_names)}
                for c in range(n_cores)
            ]

        return execute, fetch

    def run(in_maps):
        execute, fetch = prepare(in_maps)
        return fetch(execute())

    run.prepare = prepare
    return run


def _swizzle(WT, D):
    c = D // P
    return np.ascontiguousarray(
        WT.reshape(c, P, c, P).transpose(2, 1, 0, 3).reshape(D, D))


def _prep_in_maps(inputs):
    f = [np.asarray(inputs[f"f{i+1}"], dtype=np.float32) for i in range(3)]
    shared = {}
    for i in range(3):
        shared[f"Wp_s{i}"] = _swizzle(np.asarray(inputs[f"Wp{i+1}"]).T, D)
        shared[f"Wq_s{i}"] = _swizzle(np.asarray(inputs[f"Wq{i+1}"]).T, D)
        shared[f"Wk_s{i}"] = _swizzle(np.asarray(inputs[f"Wk{i+1}"]).T, D)
        shared[f"WvT{i}"] = np.ascontiguousarray(np.asarray(inputs[f"Wv{i+1}"]).T)
        shared[f"bp{i}"] = np.asarray(inputs[f"bp{i+1}"], dtype=np.float32)
    in_maps = []
    for c in range(B):
        m = dict(shared)
        for i in range(3):
            m[f"fT{i}"] = np.ascontiguousarray(f[i][c].T)
        in_maps.append(m)
    return in_maps


def get_runner(reps=1, mm_dtype_name=MM_DTYPE_NAME):
    key = (reps, mm_dtype_name)
    if key not in _STATE:
        nc = build_nc(reps=reps, mm_dtype_name=mm_dtype_name)
        _STATE[key] = _make_runner(nc)
    return _STATE[key]


def kernel(**inputs):
    run = get_runner()
    in_maps = _prep_in_maps(inputs)
    results = run(in_maps)
    out = np.stack([results[c]["out"] for c in range(B)], axis=0)
    return out


# revision 5
# speedup vs baseline: 1.0557x; 1.0557x over previous
"""CrossModalAttention on 8 Trainium2 NeuronCores (Bass/Tile, SPMD).

Sharding: data-parallel over batch B=8 (one batch element per core),
weights replicated. Each core computes, for its batch element:

  fp_i = relu(f_i @ Wp_i^T + bp_i)            i in {1,2,3}
  q_i, k_i = fp_i @ Wq_i^T, fp_i @ Wk_i^T ;  v_i = fp_i @ Wv_i^T
  s1 = (q2+q3) k1^T ; c13 = q1 k3^T ; s2 = c13 + q3 k2^T ; s3 = c13 + q2 k3^T
  u_i = softmax(s_i) v_i
  out = concat([u1, u2, u3, fp1, fp2, fp3], -1)

Device layout: activations live transposed ([D, T], contraction dim on
SBUF partitions); scores come out natural [q, k] for free-axis softmax;
p is PE-transposed for the AV matmul. Matmuls run as float32r (fp32 data,
single-pass PE mode); softmax & accumulation in fp32.

Host pre-layout: f_i^T, and the q/k/p weights pre-swizzled so each
stationary [128,128] lhsT chunk is a contiguous DRAM row-block.
"""

import math
from contextlib import ExitStack

import numpy as np

FP = None  # set in _lazy_imports
_STATE = {}

B, T, D = 8, 1024, 1024
P = 128
MM_DTYPE_NAME = "float32r"


def _lazy_imports():
    global tile, bacc, mybir, bass, make_identity, FP, AF
    import concourse.bass as bass
    import concourse.tile as tile
    from concourse import bacc, mybir
    from concourse.masks import make_identity
    FP = mybir.dt.float32
    AF = mybir.ActivationFunctionType


def build_nc(T=T, D=D, reps=1, mm_dtype_name=MM_DTYPE_NAME):
    _lazy_imports()
    mm_dtype = getattr(mybir.dt, mm_dtype_name)
    MDT = mm_dtype
    tr_dtype = mm_dtype
    Cd = D // P
    Ct = T // P
    S = min(512, T)
    NS = T // S
    Sv = min(512, D)
    NSv = D // Sv

    nc = bacc.Bacc("TRN2", target_bir_lowering=False, debug=False,
                   enable_asserts=False, num_devices=8)

    fT, Wp_s, Wq_s, Wk_s, WvT, bp = [], [], [], [], [], []
    qTd, vd, pTd = [], [], []
    for i in range(3):
        fT.append(nc.dram_tensor(f"fT{i}", [D, T], MDT, kind="ExternalInput").ap())
        Wp_s.append(nc.dram_tensor(f"Wp_s{i}", [D, D], MDT, kind="ExternalInput").ap())
        Wq_s.append(nc.dram_tensor(f"Wq_s{i}", [D, D], MDT, kind="ExternalInput").ap())
        Wk_s.append(nc.dram_tensor(f"Wk_s{i}", [D, D], MDT, kind="ExternalInput").ap())
        WvT.append(nc.dram_tensor(f"WvT{i}", [D, D], MDT, kind="ExternalInput").ap())
        bp.append(nc.dram_tensor(f"bp{i}", [D], FP, kind="ExternalInput").ap())
        qTd.append(nc.dram_tensor(f"qTd{i}", [D, T], MDT, kind="Internal").ap())
        vd.append(nc.dram_tensor(f"vd{i}", [T, D], MDT, kind="Internal").ap())
        pTd.append(nc.dram_tensor(f"pTd{i}", [Ct, T, P], MDT, kind="Internal").ap())
    out = nc.dram_tensor("out", [T, 6 * D], FP, kind="ExternalOutput").ap()

    def mm(ps, lhsT, rhs, start, stop):
        nc.tensor.matmul(ps, lhsT, rhs, start=start, stop=stop)

    with tile.TileContext(nc) as tc:
        with ExitStack() as top:
            const = top.enter_context(tc.tile_pool(name="const", bufs=1))
            ident_f = const.tile([P, P], FP)
            make_identity(nc, ident_f[:])
            ident = const.tile([P, P], tr_dtype)
            nc.vector.tensor_copy(ident[:], ident_f[:])
            bias_t = const.tile([P, 3 * Cd], FP)
            for i in range(3):
                nc.sync.dma_start(bias_t[:, i * Cd:(i + 1) * Cd],
                                  bp[i].rearrange("(c p) -> p c", p=P))
            for _rep in range(reps):
                _emit_body(nc, tc, mm, ident, bias_t,
                           fT, Wp_s, Wq_s, Wk_s, WvT, qTd, vd, pTd, out,
                           Cd, Ct, S, NS, Sv, NSv, tr_dtype, MDT, D, T)
    nc.compile()
    return nc


def _emit_body(nc, tc, mm, ident, bias_t, fT, Wp_s, Wq_s, Wk_s, WvT,
               qTd, vd, pTd, out, Cd, Ct, S, NS, Sv, NSv, tr_dtype, MDT, D, T):
    kt_tiles = [[None] * Cd for _ in range(3)]
    with ExitStack() as sAB1:
        ktp = sAB1.enter_context(tc.tile_pool(name="kt", bufs=3 * Cd))

        # ---------------- Stage A: projections, q/v spill, kT resident ----
        with ExitStack() as sA:
            ftp = sA.enter_context(tc.tile_pool(name="ft", bufs=Cd))
            fptp = sA.enter_context(tc.tile_pool(name="fpt", bufs=Cd))
            wsp = sA.enter_context(tc.tile_pool(name="wstream", bufs=2))
            wvp = sA.enter_context(tc.tile_pool(name="wv", bufs=Cd))
            qst = sA.enter_context(tc.tile_pool(name="qstage", bufs=2))
            vst = sA.enter_context(tc.tile_pool(name="vstage", bufs=2))
            fst = sA.enter_context(tc.tile_pool(name="fpstage", bufs=2))
            psA = sA.enter_context(tc.tile_pool(name="psA", bufs=6, space="PSUM"))
            psT = sA.enter_context(tc.tile_pool(name="psT", bufs=2, space="PSUM"))

            for i in range(3):
                ft_t = []
                for dc in range(Cd):
                    t = ftp.tile([P, T], MDT, tag="ft", name="ft")
                    nc.sync.dma_start(t[:], fT[i][dc * P:(dc + 1) * P, :])
                    ft_t.append(t)
                # fpT_i = relu(Wp f^T + b)
                fpt_t = [fptp.tile([P, T], MDT, tag="fpt", name="fpt")
                         for _ in range(Cd)]
                for ec in range(Cd):
                    w = wsp.tile([P, Cd * P], MDT, tag="w")
                    nc.sync.dma_start(w[:], Wp_s[i][ec * P:(ec + 1) * P, :])
                    for tn in range(NS):
                        ps = psA.tile([P, S], FP, tag="psA")
                        for dc in range(Cd):
                            mm(ps[:], w[:, dc * P:(dc + 1) * P],
                               ft_t[dc][:, tn * S:(tn + 1) * S],
                               dc == 0, dc == Cd - 1)
                        nc.scalar.activation(
                            fpt_t[ec][:, tn * S:(tn + 1) * S], ps[:], AF.Relu,
                            bias=bias_t[:, i * Cd + ec:i * Cd + ec + 1])
                # fp_i natural -> out[:, 3D + i*D ...]
                for tb in range(Ct):
                    fs = fst.tile([P, D], FP, tag="fps")
                    for ec in range(Cd):
                        pt = psT.tile([P, P], tr_dtype, tag="psT")
                        nc.tensor.transpose(
                            pt[:],
                            fpt_t[ec][:, tb * P:(tb + 1) * P],
                            ident[:])
                        nc.scalar.copy(
                            fs[:, ec * P:(ec + 1) * P], pt[:].bitcast(FP))
                    nc.sync.dma_start(
                        out[tb * P:(tb + 1) * P, 3 * D + i * D:3 * D + (i + 1) * D],
                        fs[:])
                # qT_i -> DRAM
                for oc in range(Cd):
                    w = wsp.tile([P, Cd * P], MDT, tag="w")
                    nc.sync.dma_start(w[:], Wq_s[i][oc * P:(oc + 1) * P, :])
                    qs = qst.tile([P, T], MDT, tag="qs")
                    for tn in range(NS):
                        ps = psA.tile([P, S], FP, tag="psA")
                        for ec in range(Cd):
                            mm(ps[:], w[:, ec * P:(ec + 1) * P],
                               fpt_t[ec][:, tn * S:(tn + 1) * S],
                               ec == 0, ec == Cd - 1)
                        nc.vector.tensor_copy(qs[:, tn * S:(tn + 1) * S], ps[:])
                    nc.sync.dma_start(qTd[i][oc * P:(oc + 1) * P, :], qs[:])
                # v_i -> DRAM (Wv^T streamed by output-column block)
                for en in range(NSv):
                    wv_t = []
                    for ec in range(Cd):
                        t = wvp.tile([P, Sv], MDT, tag="wv", name="wv")
                        nc.sync.dma_start(
                            t[:], WvT[i][ec * P:(ec + 1) * P, en * Sv:(en + 1) * Sv])
                        wv_t.append(t)
                    for tb in range(Ct):
                        ps = psA.tile([P, Sv], FP, tag="psA")
                        for ec in range(Cd):
                            mm(ps[:], fpt_t[ec][:, tb * P:(tb + 1) * P],
                               wv_t[ec][:], ec == 0, ec == Cd - 1)
                        vs = vst.tile([P, Sv], MDT, tag="vs")
                        nc.vector.tensor_copy(vs[:], ps[:])
                        nc.sync.dma_start(
                            vd[i][tb * P:(tb + 1) * P, en * Sv:(en + 1) * Sv], vs[:])
                # kT_i -> SBUF resident
                for oc in range(Cd):
                    w = wsp.tile([P, Cd * P], MDT, tag="w")
                    nc.sync.dma_start(w[:], Wk_s[i][oc * P:(oc + 1) * P, :])
                    kt = ktp.tile([P, T], MDT, tag="kt")
                    for tn in range(NS):
                        ps = psA.tile([P, S], FP, tag="psA")
                        for ec in range(Cd):
                            mm(ps[:], w[:, ec * P:(ec + 1) * P],
                               fpt_t[ec][:, tn * S:(tn + 1) * S],
                               ec == 0, ec == Cd - 1)
                        nc.scalar.copy(kt[:, tn * S:(tn + 1) * S], ps[:])
                    kt_tiles[i][oc] = kt

        # ---------------- Stage B1: scores + softmax + p^T spill ----------
        with ExitStack() as sB1:
            qlp = sB1.enter_context(tc.tile_pool(name="qload", bufs=8))
            ppp = sB1.enter_context(tc.tile_pool(name="pp", bufs=3))
            ptp = sB1.enter_context(tc.tile_pool(name="ptstage", bufs=3))
            c13p = sB1.enter_context(tc.tile_pool(name="c13", bufs=2))
            stp = sB1.enter_context(tc.tile_pool(name="stats", bufs=2))
            psS = sB1.enter_context(tc.tile_pool(name="psS", bufs=3, space="PSUM"))
            psT2 = sB1.enter_context(tc.tile_pool(name="psT2", bufs=2, space="PSUM"))

            for qb in range(Ct):
                qt = []
                for i in range(3):
                    q = qlp.tile([P, Cd * P], MDT, tag="q", name="q")
                    nc.sync.dma_start(
                        q[:].rearrange("p (c t) -> p c t", c=Cd),
                        qTd[i].rearrange("(c p) t -> p c t", p=P)
                           [:, :, qb * P:(qb + 1) * P])
                    qt.append(q)
                qsum = qlp.tile([P, Cd * P], MDT, tag="q", name="qsum")
                nc.vector.tensor_tensor(qsum[:], qt[1][:], qt[2][:],
                                        mybir.AluOpType.add)
                stats = stp.tile([P, 16], FP, tag="stats")

                def softmax(s_ps, i_mod, col, qb=qb, stats=stats):
                    negmax = stats[:, col:col + 1]
                    denom = stats[:, col + 4:col + 5]
                    recip = stats[:, col + 8:col + 9]
                    nc.vector.tensor_reduce(negmax, s_ps[:], mybir.AxisListType.X,
                                            mybir.AluOpType.max, negate=True)
                    p = ppp.tile([P, T], MDT, tag="p", name="p")
                    nc.scalar.activation(p[:], s_ps[:], AF.Exp,
                                         bias=negmax, accum_out=denom)
                    nc.vector.reciprocal(recip, denom)
                    nc.vector.tensor_scalar_mul(p[:], p[:], recip)
                    pts = ptp.tile([P, T], MDT, tag="pts", name="pts")
                    for kc in range(Ct):
                        pt = psT2.tile([P, P], tr_dtype, tag="psT2", name="pt")
                        nc.tensor.transpose(
                            pt[:], p[:, kc * P:(kc + 1) * P],
                            ident[:])
                        nc.vector.tensor_copy(
                            pts[:, kc * P:(kc + 1) * P], pt[:])
                    nc.sync.dma_start(
                        pTd[i_mod][qb].rearrange("(kc p) q -> p kc q", p=P),
                        pts[:].rearrange("p (kc q) -> p kc q", q=P))

                # s1 = (q2+q3) k1^T
                s1 = psS.tile([P, T], FP, tag="s", name="s1")
                for nh in range(NS):
                    for ec in range(Cd):
                        mm(s1[:, nh * S:(nh + 1) * S], qsum[:, ec * P:(ec + 1) * P],
                           kt_tiles[0][ec][:, nh * S:(nh + 1) * S], ec == 0,
                           ec == Cd - 1)
                softmax(s1, 0, 0)
                # c13 = q1 k3^T
                c13ps = psS.tile([P, T], FP, tag="s", name="c13ps")
                for nh in range(NS):
                    for ec in range(Cd):
                        mm(c13ps[:, nh * S:(nh + 1) * S],
                           qt[0][:, ec * P:(ec + 1) * P],
                           kt_tiles[2][ec][:, nh * S:(nh + 1) * S], ec == 0,
                           ec == Cd - 1)
                c13 = c13p.tile([P, T], FP, tag="c13", name="c13")
                nc.vector.tensor_copy(c13[:], c13ps[:])
                # s2 = c13 + q3 k2^T
                s2 = psS.tile([P, T], FP, tag="s", name="s2")
                for nh in range(NS):
                    for ec in range(Cd):
                        mm(s2[:, nh * S:(nh + 1) * S], qt[2][:, ec * P:(ec + 1) * P],
                           kt_tiles[1][ec][:, nh * S:(nh + 1) * S], ec == 0,
                           ec == Cd - 1)
                nc.vector.tensor_tensor(s2[:], s2[:], c13[:], mybir.AluOpType.add)
                softmax(s2, 1, 1)
                # s3 = c13 + q2 k3^T
                s3 = psS.tile([P, T], FP, tag="s", name="s3")
                for nh in range(NS):
                    for ec in range(Cd):
                        mm(s3[:, nh * S:(nh + 1) * S], qt[1][:, ec * P:(ec + 1) * P],
                           kt_tiles[2][ec][:, nh * S:(nh + 1) * S], ec == 0,
                           ec == Cd - 1)
                nc.vector.tensor_tensor(s3[:], s3[:], c13[:], mybir.AluOpType.add)
                softmax(s3, 2, 2)

    # ---------------- Stage B2: u = p v -------------------------------
    with ExitStack() as sB2:
        vlp = sB2.enter_context(tc.tile_pool(name="vload", bufs=3 * Ct))
        ptl = sB2.enter_context(tc.tile_pool(name="ptload", bufs=6))
        ust = sB2.enter_context(tc.tile_pool(name="ustage", bufs=3))
        psU = sB2.enter_context(tc.tile_pool(name="psU", bufs=3, space="PSUM"))

        v_tiles = [[None] * Ct for _ in range(3)]
        for i in range(3):
            for kc in range(Ct):
                t = vlp.tile([P, D], MDT, tag="v", name="v")
                nc.sync.dma_start(t[:], vd[i][kc * P:(kc + 1) * P, :])
                v_tiles[i][kc] = t
        for qb in range(Ct):
            for i in range(3):
                ptile = ptl.tile([P, T], MDT, tag="pt", name="ptile")
                nc.sync.dma_start(
                    ptile[:].rearrange("p (kc q) -> p kc q", q=P),
                    pTd[i][qb].rearrange("(kc p) q -> p kc q", p=P))
                psu = psU.tile([P, D], FP, tag="psu")
                for en in range(NSv):
                    for kc in range(Ct):
                        mm(psu[:, en * Sv:(en + 1) * Sv],
                           ptile[:, kc * P:(kc + 1) * P],
                           v_tiles[i][kc][:, en * Sv:(en + 1) * Sv],
                           kc == 0, kc == Ct - 1)
                us = ust.tile([P, D], FP, tag="us")
                nc.scalar.copy(us[:], psu[:])
                nc.sync.dma_start(out[qb * P:(qb + 1) * P, i * D:(i + 1) * D], us[:])


# ---------------------------------------------------------------------------
# Host side: runner + kernel()
# ---------------------------------------------------------------------------

def _make_runner(nc, n_cores=8):
    import jax
    from jax.sharding import Mesh, PartitionSpec
    from jax.experimental.shard_map import shard_map
    from concourse import mybir
    from concourse.bass2jax import (_bass_exec_p, install_neuronx_cc_hook,
                                    partition_id_tensor)

    install_neuronx_cc_hook()
    partition_name = (nc.partition_id_tensor.name
                      if nc.partition_id_tensor else None)
    in_names, out_names, out_avals, zero_outs = [], [], [], []
    for alloc in nc.m.functions[0].allocations:
        if not isinstance(alloc, mybir.MemoryLocationSet):
            continue
        name = alloc.memorylocations[0].name
        if alloc.kind == "ExternalInput":
            if name != partition_name:
                in_names.append(name)
        elif alloc.kind == "ExternalOutput":
            out_names.append(name)
            shape = tuple(alloc.tensor_shape)
            dtype = mybir.dt.np(alloc.dtype)
            out_avals.append(jax.core.ShapedArray(shape, dtype))
            zero_outs.append(np.zeros(shape, dtype))
    n_params = len(in_names)
    all_names = in_names + out_names
    if partition_name is not None:
        all_names.append(partition_name)

    def _body(*args):
        operands = list(args)
        if partition_name is not None:
            operands.append(partition_id_tensor())
        outs = _bass_exec_p.bind(
            *operands,
            out_avals=tuple(out_avals),
            in_names=tuple(all_names),
            out_names=tuple(out_names),
            lowering_input_output_aliases=(),
            sim_require_finite=True,
            sim_require_nnan=True,
            nc=nc,
        )
        return tuple(outs)

    devices = jax.devices()[:n_cores]
    mesh = Mesh(np.asarray(devices), ("core",))
    specs = (PartitionSpec("core"),)
    sharded = jax.jit(
        shard_map(_body, mesh=mesh,
                  in_specs=specs * (n_params + len(out_names)),
                  out_specs=specs * len(out_names), check_rep=False),
        keep_unused=True,
    )
    sharding = jax.sharding.NamedSharding(mesh, PartitionSpec("core"))

    def prepare(in_maps):
        per_core = [[np.asarray(m[name]) for name in in_names] for m in in_maps]
        concat_in = [np.concatenate([per_core[c][i] for c in range(n_cores)],
                                    axis=0) for i in range(n_params)]
        concat_zeros = [np.zeros((n_cores * z.shape[0], *z.shape[1:]), z.dtype)
                        for z in zero_outs]
        dev_in = [jax.device_put(a, sharding) for a in concat_in]
        dev_zero = [jax.device_put(a, sharding) for a in concat_zeros]
        jax.block_until_ready(dev_in)
        jax.block_until_ready(dev_zero)

        def execute():
            out = sharded(*dev_in, *dev_zero)
            jax.block_until_ready(out)
            return out

        def fetch(out):
            return [
                {name: np.asarray(out[i]).reshape(n_cores, *out_avals[i].shape)[c]
                 for i, name in enumerate(out_names)}
                for c in range(n_cores)
            ]

        return execute, fetch

    def run(in_maps):
        execute, fetch = prepare(in_maps)
        return fetch(execute())

    run.prepare = prepare
    return run


def _swizzle(WT, D):
    c = D // P
    return np.ascontiguousarray(
        WT.reshape(c, P, c, P).transpose(2, 1, 0, 3).reshape(D, D))


def _prep_in_maps(inputs):
    f = [np.asarray(inputs[f"f{i+1}"], dtype=np.float32) for i in range(3)]
    shared = {}
    for i in range(3):
        shared[f"Wp_s{i}"] = _swizzle(np.asarray(inputs[f"Wp{i+1}"]).T, D)
        shared[f"Wq_s{i}"] = _swizzle(np.asarray(inputs[f"Wq{i+1}"]).T, D)
        shared[f"Wk_s{i}"] = _swizzle(np.asarray(inputs[f"Wk{i+1}"]).T, D)
        shared[f"WvT{i}"] = np.ascontiguousarray(np.asarray(inputs[f"Wv{i+1}"]).T)
        shared[f"bp{i}"] = np.asarray(inputs[f"bp{i+1}"], dtype=np.float32)
    in_maps = []
    for c in range(B):
        m = dict(shared)
        for i in range(3):
            m[f"fT{i}"] = np.ascontiguousarray(f[i][c].T)
        in_maps.append(m)
    return in_maps


def get_runner(reps=1, mm_dtype_name=MM_DTYPE_NAME):
    key = (reps, mm_dtype_name)
    if key not in _STATE:
        nc = build_nc(reps=reps, mm_dtype_name=mm_dtype_name)
        _STATE[key] = _make_runner(nc)
    return _STATE[key]


def kernel(**inputs):
    run = get_runner()
    in_maps = _prep_in_maps(inputs)
    results = run(in_maps)
    out = np.stack([results[c]["out"] for c in range(B)], axis=0)
    return out
